# revision 44
# baseline (speedup 1.0000x reference)
"""GQA attention (B=2, L=2048, E=2048, 32 q-heads / 8 kv-heads, D=64) on 8 trn2
NeuronCores.

Sharding: tensor-parallel over kv-heads. Core h owns kv-head h: the 4 q-heads
4h..4h+3 (W_Q rows 256h:256h+256), W_K/W_V rows 64h:64h+64, and W_O columns
256h:256h+256. Each core computes a full-shape partial output (bf16); the host
sums the 8 partials (the "all-reduce") and transposes back.

Perf design (971us baseline -> ~461us):
  - All matmul operands are bf16 (fp32 PSUM accumulate): halves HBM/SBUF
    traffic, enables fast weight load, and keeps the HAM clock-gate at
    K=8/8 (the fp32r baseline spent 790us throttled to 1.2GHz).
  - Scores (contraction = head_dim = 64) use PE row tiling: the q-head pair
    (h at partitions 0:64, h' at 64:128 — the natural projection layout) runs
    as two concurrent matmuls at tile_position (0,0)/(64,0) against K^T and a
    partition-64:128 duplicate of K^T, writing the two halves of a
    [128, 1024] PSUM tile (two banks). 2x scores throughput and no odd-head
    Q restage. One ACT instruction exps both heads' tiles (scale=0.125 fused;
    no max subtraction needed — scores ~ N(0,1)).
  - QKV projection matmuls are split into contraction row-halves feeding two
    PSUM banks at alternating row groups (the halves stream concurrently and
    each LDWEIGHTS hides under the other half's matmul); the DVE merge
    (copy high + in-place add low) replaces the PSUM->SBUF copy.
  - es tiles for a whole (q-chunk, pair) stay resident (16 x [128,1024] bf16)
    so both heads' attn@V accumulate kt-major with dense PE work; the
    denominator comes from a ones-column appended to V^T (M = 65). v_sb
    chunks are padded to 66 columns so the ones-DMA and the V-transpose DVE
    copies never write bf16 halves of the same 4-byte SBUF word (sub-word
    RMW race, intermittent corruption otherwise).
  - attn@V is software-pipelined ONE PAIR behind scores: each kt slot emits
    [scores(p,kt) pair | exp(p,kt) | attn@V(p-1,kt) x2 | one deferred unit],
    keeping the PE stream in lockstep with ACT's exp cadence (scores stall
    on the 2-deep PSUM ring behind ACT anyway; without this the attn@V tail
    added ~3.5us per pair). attn@V results are copied to SBUF immediately
    (frees the PSUM bank); the normalize chain (denom row to partition 0 —
    the custom DVE reciprocal_approx_fast only works at base 0 — ones-matmul
    broadcast, mul) and the output projection are the deferred units.
  - Batch 1's x tiles prefetch during batch 0's attention; output is staged
    to bf16 and DMA'd per 512x512 block; the host sums the 8 bf16 partials.
"""

import numpy as np

B, L, E = 2, 2048, 2048
HKV, D, G = 8, 64, 4          # kv heads (=cores), head dim, q-heads per core
QD = G * D                    # 256 q dims per core
N_CORES = 8
EC = E // 128                 # 16 contraction chunks for projections
NT = L // 512                 # 4 token chunks of 512
KT = L // 128                 # 16 k-token tiles of 128

_cache = {}


def _build_nc():
    import concourse.bass as bass
    import concourse.mybir as mybir
    import concourse.tile as tile
    from concourse import bacc
    from contextlib import ExitStack

    f32 = mybir.dt.float32
    bf16 = mybir.dt.bfloat16

    nc = bacc.Bacc("TRN2", target_bir_lowering=False, debug=False)
    xT_d = nc.declare_dram_parameter("xT", [B, E, L], bf16, isOutput=False)
    wq_d = nc.declare_dram_parameter("wq", [E, QD], bf16, isOutput=False)
    wkv_d = nc.declare_dram_parameter("wkv", [E, 2 * D], bf16, isOutput=False)
    wo_d = nc.declare_dram_parameter("wo", [QD, E], bf16, isOutput=False)
    ident_d = nc.declare_dram_parameter("ident", [128, 128], bf16, isOutput=False)
    ones_d = nc.declare_dram_parameter("ones", [1, 128], bf16, isOutput=False)
    out_d = nc.declare_dram_parameter("out", [B, E, L], bf16, isOutput=True)

    with ExitStack() as ctx:
        tc = ctx.enter_context(tile.TileContext(nc))
        singles = ctx.enter_context(tc.tile_pool(name="singles", bufs=1))
        xt_pool = ctx.enter_context(tc.tile_pool(name="xtp", bufs=16))
        qt_pool = ctx.enter_context(tc.tile_pool(name="qtp", bufs=2))
        kv_pool = ctx.enter_context(tc.tile_pool(name="kvp", bufs=2))
        kd_pool = ctx.enter_context(tc.tile_pool(name="kdp", bufs=2))
        vsb_pool = ctx.enter_context(tc.tile_pool(name="vsp", bufs=2))
        es_pool = ctx.enter_context(tc.tile_pool(name="esp", bufs=21))
        ot_pool = ctx.enter_context(tc.tile_pool(name="otp", bufs=2))
        vo_pool = ctx.enter_context(tc.tile_pool(name="vop", bufs=2))
        rec_pool = ctx.enter_context(tc.tile_pool(name="rcp", bufs=2))
        ntmp_pool = ctx.enter_context(tc.tile_pool(name="ntp", bufs=2))
        stage_pool = ctx.enter_context(tc.tile_pool(name="stp", bufs=2))
        ps_mm = ctx.enter_context(tc.tile_pool(name="psmm", bufs=2, space="PSUM"))
        ps_sc = ctx.enter_context(tc.tile_pool(name="pssc", bufs=2, space="PSUM"))
        ps_va = ctx.enter_context(tc.tile_pool(name="psva", bufs=2, space="PSUM"))

        # ---- static weights / constants ----
        # wkv first (the first QKV group needs it); wq/wo are DMA'd after the
        # batch-0 x tiles so they don't delay the first matmuls
        wkv_sb = singles.tile([128, EC * 2 * D], bf16)
        nc.sync.dma_start(
            out=wkv_sb.rearrange("p (e m) -> p e m", e=EC),
            in_=wkv_d.rearrange("(e p) m -> p e m", p=128),
        )
        ident = singles.tile([128, 128], bf16)
        nc.sync.dma_start(out=ident, in_=ident_d[:, :])
        wq_sb = singles.tile([128, EC * QD], bf16)  # e-chunk e at cols [e*256,(e+1)*256)
        wo_sb = [singles.tile([128, E], bf16, name=f"wo_sb{kc}")
                 for kc in range(2)]
        # ones on every partition (the broadcast matmul's lhsT sits at
        # partition 64, next to the denominator row)
        onesP = singles.tile([128, 64], bf16)
        nc.vector.memset(onesP, 1.0)

        units = []  # deferred normalize / O-proj units, popped between matmuls
        prev_pair = None

        def emit_x_loads(b_):
            tiles = []
            for e in range(EC):
                xt = xt_pool.tile([128, L], bf16, name=f"xt_{e}", tag="xt")
                nc.sync.dma_start(out=xt,
                                  in_=xT_d[b_, e * 128:(e + 1) * 128, :])
                tiles.append(xt)
            return tiles

        xts_next = {}
        for b in range(B):
            # ---- x load: full batch, 16 e-chunk tiles ----
            xts = xts_next.pop(b) if b in xts_next else emit_x_loads(b)

            # ---- QKV projections (kv first so kdup/v_sb prep overlaps) ----
            qpair = [qt_pool.tile([128, L], bf16, name=f"qpair{p}", tag=f"qpair{p}")
                     for p in range(2)]
            kvT = kv_pool.tile([128, L], bf16, name="kvT")  # K^T rows 0:64, V^T rows 64:128
            kdup = kd_pool.tile([128, L], bf16, name="kdup")  # K^T dup at rows 64:128
            # chunk stride 66 = [V (64) | ones | pad] keeps the DMA- and
            # DVE-written bf16 regions 4-byte-word-disjoint
            VW = D + 2
            v_sb = vsb_pool.tile([128, KT * VW], bf16, name="v_sb")

            def make_qkv_group(m, t, xts_, qpair_, kvT_):
                def emit():
                    pp = ps_sc.tile([128, 1024], f32, name="ps_qkv", tag="sc")
                    for e in range(EC):
                        if m < 2:
                            lhsT = wq_sb[:, e * QD + m * 128:
                                         e * QD + (m + 1) * 128]
                        else:
                            lhsT = wkv_sb[:, e * 2 * D:(e + 1) * 2 * D]
                        for hf in range(2):
                            hs = slice(hf * 64, (hf + 1) * 64)
                            nc.tensor.matmul(
                                pp[:, hf * 512:(hf + 1) * 512],
                                lhsT[hs, :],
                                xts_[e][hs, t * 512:(t + 1) * 512],
                                start=(e == 0), stop=(e == EC - 1),
                            )
                    dst = qpair_[m] if m < 2 else kvT_
                    dsl = dst[:, t * 512:(t + 1) * 512]
                    nc.vector.tensor_copy(dsl, pp[:, 512:1024])
                    nc.vector.tensor_add(dsl, dsl, pp[:, 0:512])
                return emit

            # batch 0: heads 2,3 projection is deferred into the attention
            # unit queue — it backfills the software pipeline's warmup idle
            # (pair 0 has no prev-attnV, pair 1 has no unit backlog yet)
            ms = (2, 0) if b == 0 else (2, 0, 1)
            for m in ms:  # kv first, then q head pairs
                for t in range(NT):
                    make_qkv_group(m, t, xts, qpair, kvT)()
                    if units:  # drain previous batch's tail work
                        units.pop(0)()

                if m == 2:
                    # K^T duplicate for the row-tiled scores matmul, and the
                    # ones column of v_sb (denominator trick) — both ahead of
                    # the deferred q/o weight loads in the DMA queue so the
                    # first attention pair isn't blocked
                    nc.sync.dma_start(out=kdup[64:128, :], in_=kvT[0:64, :])
                    ones_bcast = bass.AP(
                        tensor=ones_d[0:1, 0:KT].tensor, offset=0,
                        ap=[[0, 128], [1, KT]])
                    nc.sync.dma_start(
                        out=v_sb.rearrange("p (k c) -> p k c", c=VW)[:, :, D],
                        in_=ones_bcast)
                    if b == 0:
                        nc.sync.dma_start(
                            out=wq_sb.rearrange("p (e m) -> p e m", e=EC),
                            in_=wq_d.rearrange("(e p) m -> p e m", p=128),
                        )
                        for kc in range(2):
                            nc.sync.dma_start(
                                out=wo_sb[kc],
                                in_=wo_d[kc * 128:(kc + 1) * 128, :])
                    # V transpose: (d, tok) -> v_sb (tok, d | ones) blocks.
                    # Done as a normal matmul (V^T_chunk).T @ I_64 so the
                    # PSUM output stays fp32 (bf16 transpose-mode PSUM writes
                    # are broken on TRN2).
                    for kt in range(KT):
                        psv = ps_mm.tile([128, 64], f32, name="ps_vt", tag="mm")
                        nc.tensor.matmul(
                            psv, kvT[64:128, kt * 128:(kt + 1) * 128],
                            ident[64:128, 64:128],
                            start=True, stop=True,
                        )
                        nc.vector.tensor_copy(
                            v_sb[:, kt * VW: kt * VW + D], psv
                        )

            # ---- attention + interleaved output projection ----
            # Normalize + O-proj are deferred into a queue of small units
            # (1-2 PE matmuls each) popped between attn@V steps, so the PE
            # always has ready work while ACT crawls through the exps.
            outT = [ot_pool.tile([128, L], bf16, name=f"outT{p}", tag=f"outT{p}")
                    for p in range(2)]

            def make_normalize(vo, m, odd, qsl, outT_=None):
                outT_ = outT if outT_ is None else outT_
                def emit():
                    # denom row to partition 0 (shifted copies are legal,
                    # the custom approx op only works at base 0)
                    dcp = rec_pool.tile([1, 512], f32, name="dcp", tag="dcp")
                    nc.vector.tensor_copy(dcp, vo[D:D + 1, :])
                    recT = rec_pool.tile([1, 512], f32, name="recT", tag="rec")
                    nc.vector.reciprocal_approx_fast(out=recT, in_=dcp)
                    recB = rec_pool.tile([1, 512], bf16, name="recB",
                                         tag="recB")
                    nc.vector.tensor_copy(recB, recT)
                    bc = ps_mm.tile([64, 512], f32, name="ps_bc", tag="mm")
                    nc.tensor.matmul(
                        bc, onesP[0:1, 0:64], recB, start=True, stop=True,
                    )
                    if not odd:
                        nc.vector.tensor_mul(
                            outT_[m][0:64, qsl], vo[0:D, :], bc
                        )
                    else:
                        ntmp = ntmp_pool.tile([64, 512], bf16, name="ntmp",
                                              tag="ntmp")
                        nc.vector.tensor_mul(ntmp, vo[0:D, :], bc)
                        nc.sync.dma_start(out=outT_[m][64:128, qsl],
                                           in_=ntmp)
                return emit

            def make_oproj_units(b_, qc, outT_=None):
                outT_ = outT if outT_ is None else outT_
                qsl = slice(qc * 512, (qc + 1) * 512)
                sts = {}
                units = []
                for m4 in range(4):
                    for mi in range(4):
                        def u(m4=m4, mi=mi):
                            if mi == 0:
                                sts[m4] = stage_pool.tile(
                                    [128, 2048], bf16, name="st", tag="st")
                            st = sts[m4]
                            mc = m4 * 4 + mi
                            msl = slice(mc * 128, (mc + 1) * 128)
                            ps = ps_mm.tile([128, 512], f32, name="ps_op",
                                            tag="mm")
                            for kc in range(2):
                                nc.tensor.matmul(
                                    ps, wo_sb[kc][:, msl], outT_[kc][:, qsl],
                                    start=(kc == 0), stop=(kc == 1),
                                )
                            nc.vector.tensor_copy(
                                st[:, mi * 512:(mi + 1) * 512], ps)
                            if mi == 3:
                                nc.sync.dma_start(
                                    out=out_d[b_, 512 * m4:512 * (m4 + 1),
                                              qsl].rearrange(
                                        "(c p) q -> p c q", c=4),
                                    in_=st.rearrange("p (c q) -> p c q", c=4),
                                )
                        units.append(u)
                return units

            if b == 0:
                for t in range(NT):
                    units.append(make_qkv_group(1, t, xts, qpair, kvT))

            def finish_pair(pv, vaccs):
                # vo copies free the PSUM banks; normalize + O-proj deferred
                for odd in range(2):
                    vo = vo_pool.tile([128, 512], f32, name=f"vo{odd}",
                                      tag=f"vo{odd}")
                    nc.vector.tensor_copy(vo[0:D + 1, :],
                                          vaccs[odd][0:D + 1, :])
                    units.append(make_normalize(vo, pv["m"], odd, pv["qsl"],
                                                pv["outT"]))
                if pv["m"] == 1:
                    units.extend(make_oproj_units(pv["b"], pv["qc"],
                                                  pv["outT"]))

            for qc in range(NT):
                if qc == 1 and b + 1 < B:
                    # prefetch next batch's x during this batch's attention
                    xts_next[b + 1] = emit_x_loads(b + 1)
                qsl = slice(qc * 512, (qc + 1) * 512)
                for m in range(2):  # head pair (h=2m at part 0:64, h'=2m+1 at 64:128)
                    # scores+exp for THIS pair, attn@V for the PREVIOUS pair,
                    # and one deferred unit share each kt slot so the PE
                    # stream stays in lockstep with ACT's exp cadence
                    pv = prev_pair
                    if pv is not None:
                        vaccs = [ps_va.tile([128, 512], f32,
                                            name=f"ps_vacc{o}", tag="vacc")
                                 for o in range(2)]
                    es_list = []
                    for kt in range(KT):
                        ksl = slice(kt * 128, (kt + 1) * 128)
                        ssc = ps_sc.tile([128, 1024], f32, name="ps_sc", tag="sc")
                        nc.tensor.matmul(
                            ssc[:, 0:512], kvT[0:64, ksl], qpair[m][0:64, qsl],
                            start=True, stop=True,
                        )
                        nc.tensor.matmul(
                            ssc[:, 512:1024], kdup[64:128, ksl],
                            qpair[m][64:128, qsl],
                            start=True, stop=True,
                        )
                        es = es_pool.tile([128, 1024], bf16, name="es", tag="es")
                        nc.scalar.activation(
                            es, ssc, mybir.ActivationFunctionType.Exp, scale=0.125
                        )
                        es_list.append(es)
                        if pv is not None:
                            for odd in range(2):
                                nc.tensor.matmul(
                                    vaccs[odd][0:D + 1, :],
                                    pv["vsb"][:, kt * VW: kt * VW + D + 1],
                                    pv["es"][kt][:, odd * 512:(odd + 1) * 512],
                                    start=(kt == 0), stop=(kt == KT - 1),
                                )
                        if units:
                            units.pop(0)()
                    if pv is not None:
                        finish_pair(pv, vaccs)
                    prev_pair = dict(es=es_list, m=m, qsl=qsl, qc=qc, b=b,
                                     vsb=v_sb, outT=outT)
        # drain the last pair's attn@V and all deferred work
        pv = prev_pair
        vaccs = [ps_va.tile([128, 512], f32, name=f"ps_vacc{o}", tag="vacc")
                 for o in range(2)]
        for kt in range(KT):
            for odd in range(2):
                nc.tensor.matmul(
                    vaccs[odd][0:D + 1, :],
                    pv["vsb"][:, kt * VW: kt * VW + D + 1],
                    pv["es"][kt][:, odd * 512:(odd + 1) * 512],
                    start=(kt == 0), stop=(kt == KT - 1),
                )
            if units:
                units.pop(0)()
        finish_pair(pv, vaccs)
        while units:
            units.pop(0)()
    nc.compile()
    return nc


def _get_nc():
    if "nc" not in _cache:
        _cache["nc"] = _build_nc()
    return _cache["nc"]


def make_in_maps(x, W_Q, W_K, W_V, W_O):
    import ml_dtypes
    bf16 = ml_dtypes.bfloat16

    x = np.asarray(x, np.float32)
    W_Q = np.asarray(W_Q, np.float32)
    W_K = np.asarray(W_K, np.float32)
    W_V = np.asarray(W_V, np.float32)
    W_O = np.asarray(W_O, np.float32)
    xT = np.ascontiguousarray(x.transpose(0, 2, 1)).astype(bf16)
    in_maps = []
    for h in range(N_CORES):
        in_maps.append({
            "xT": xT,
            "wq": np.ascontiguousarray(W_Q[QD * h:QD * (h + 1), :].T).astype(bf16),
            "wkv": np.ascontiguousarray(
                np.concatenate([W_K[D * h:D * (h + 1), :],
                                W_V[D * h:D * (h + 1), :]], axis=0).T).astype(bf16),
            "wo": np.ascontiguousarray(W_O[:, QD * h:QD * (h + 1)].T).astype(bf16),
            "ident": np.eye(128, dtype=np.float32).astype(bf16),
            "ones": np.ones((1, 128), np.float32).astype(bf16),
        })
    return in_maps


def run_spmd(x, W_Q, W_K, W_V, W_O, **spmd_kwargs):
    from concourse.bass_utils import run_bass_kernel_spmd

    nc = _get_nc()
    in_maps = make_in_maps(x, W_Q, W_K, W_V, W_O)
    res = run_bass_kernel_spmd(nc, in_maps, list(range(N_CORES)), **spmd_kwargs)
    total = np.zeros((B, E, L), np.float32)
    for r in res.results:
        total += np.asarray(r["out"]).astype(np.float32)
    out = np.ascontiguousarray(total.transpose(0, 2, 1))
    return out, res


def kernel(x, W_Q, W_K, W_V, W_O):
    out, _ = run_spmd(x, W_Q, W_K, W_V, W_O)
    return out


# revision 45
# speedup vs baseline: 1.0101x; 1.0101x over previous
"""GQA attention (B=2, L=2048, E=2048, 32 q-heads / 8 kv-heads, D=64) on 8 trn2
NeuronCores.

Sharding: tensor-parallel over kv-heads. Core h owns kv-head h: the 4 q-heads
4h..4h+3 (W_Q rows 256h:256h+256), W_K/W_V rows 64h:64h+64, and W_O columns
256h:256h+256. Each core computes a full-shape partial output (bf16); the host
sums the 8 partials (the "all-reduce") and transposes back.

Perf design (971us baseline -> ~461us):
  - All matmul operands are bf16 (fp32 PSUM accumulate): halves HBM/SBUF
    traffic, enables fast weight load, and keeps the HAM clock-gate at
    K=8/8 (the fp32r baseline spent 790us throttled to 1.2GHz).
  - Scores (contraction = head_dim = 64) use PE row tiling: the q-head pair
    (h at partitions 0:64, h' at 64:128 — the natural projection layout) runs
    as two concurrent matmuls at tile_position (0,0)/(64,0) against K^T and a
    partition-64:128 duplicate of K^T, writing the two halves of a
    [128, 1024] PSUM tile (two banks). 2x scores throughput and no odd-head
    Q restage. One ACT instruction exps both heads' tiles (scale=0.125 fused;
    no max subtraction needed — scores ~ N(0,1)).
  - QKV projection matmuls are split into contraction row-halves feeding two
    PSUM banks at alternating row groups (the halves stream concurrently and
    each LDWEIGHTS hides under the other half's matmul); the DVE merge
    (copy high + in-place add low) replaces the PSUM->SBUF copy.
  - es tiles for a whole (q-chunk, pair) stay resident (16 x [128,1024] bf16)
    so both heads' attn@V accumulate kt-major with dense PE work; the
    denominator comes from a ones-column appended to V^T (M = 65). v_sb
    chunks are padded to 66 columns so the ones-DMA and the V-transpose DVE
    copies never write bf16 halves of the same 4-byte SBUF word (sub-word
    RMW race, intermittent corruption otherwise).
  - attn@V is software-pipelined ONE PAIR behind scores: each kt slot emits
    [scores(p,kt) pair | exp(p,kt) | attn@V(p-1,kt) x2 | one deferred unit],
    keeping the PE stream in lockstep with ACT's exp cadence (scores stall
    on the 2-deep PSUM ring behind ACT anyway; without this the attn@V tail
    added ~3.5us per pair). attn@V results are copied to SBUF immediately
    (frees the PSUM bank); the normalize chain (denom row to partition 0 —
    the custom DVE reciprocal_approx_fast only works at base 0 — ones-matmul
    broadcast, mul) and the output projection are the deferred units.
  - Batch 1's x tiles prefetch during batch 0's attention; output is staged
    to bf16 and DMA'd per 512x512 block; the host sums the 8 bf16 partials.
"""

import numpy as np

B, L, E = 2, 2048, 2048
HKV, D, G = 8, 64, 4          # kv heads (=cores), head dim, q-heads per core
QD = G * D                    # 256 q dims per core
N_CORES = 8
EC = E // 128                 # 16 contraction chunks for projections
NT = L // 512                 # 4 token chunks of 512
KT = L // 128                 # 16 k-token tiles of 128

_cache = {}


def _build_nc():
    import concourse.bass as bass
    import concourse.mybir as mybir
    import concourse.tile as tile
    from concourse import bacc
    from contextlib import ExitStack

    f32 = mybir.dt.float32
    bf16 = mybir.dt.bfloat16

    nc = bacc.Bacc("TRN2", target_bir_lowering=False, debug=False)
    xT_d = nc.declare_dram_parameter("xT", [B, E, L], bf16, isOutput=False)
    wq_d = nc.declare_dram_parameter("wq", [E, QD], bf16, isOutput=False)
    wkv_d = nc.declare_dram_parameter("wkv", [E, 2 * D], bf16, isOutput=False)
    wo_d = nc.declare_dram_parameter("wo", [QD, E], bf16, isOutput=False)
    ident_d = nc.declare_dram_parameter("ident", [128, 128], bf16, isOutput=False)
    ones_d = nc.declare_dram_parameter("ones", [1, 128], bf16, isOutput=False)
    out_d = nc.declare_dram_parameter("out", [B, E, L], bf16, isOutput=True)

    with ExitStack() as ctx:
        tc = ctx.enter_context(tile.TileContext(nc))
        singles = ctx.enter_context(tc.tile_pool(name="singles", bufs=1))
        xt_pool = ctx.enter_context(tc.tile_pool(name="xtp", bufs=16))
        qt_pool = ctx.enter_context(tc.tile_pool(name="qtp", bufs=2))
        kv_pool = ctx.enter_context(tc.tile_pool(name="kvp", bufs=2))
        kd_pool = ctx.enter_context(tc.tile_pool(name="kdp", bufs=2))
        vsb_pool = ctx.enter_context(tc.tile_pool(name="vsp", bufs=2))
        es_pool = ctx.enter_context(tc.tile_pool(name="esp", bufs=21))
        ot_pool = ctx.enter_context(tc.tile_pool(name="otp", bufs=2))
        vo_pool = ctx.enter_context(tc.tile_pool(name="vop", bufs=2))
        rec_pool = ctx.enter_context(tc.tile_pool(name="rcp", bufs=2))
        ntmp_pool = ctx.enter_context(tc.tile_pool(name="ntp", bufs=2))
        stage_pool = ctx.enter_context(tc.tile_pool(name="stp", bufs=2))
        ps_mm = ctx.enter_context(tc.tile_pool(name="psmm", bufs=2, space="PSUM"))
        ps_sc = ctx.enter_context(tc.tile_pool(name="pssc", bufs=2, space="PSUM"))
        ps_va = ctx.enter_context(tc.tile_pool(name="psva", bufs=2, space="PSUM"))

        # ---- static weights / constants ----
        # wkv first (the first QKV group needs it); wq/wo are DMA'd after the
        # batch-0 x tiles so they don't delay the first matmuls
        wkv_sb = singles.tile([128, EC * 2 * D], bf16)
        nc.sync.dma_start(
            out=wkv_sb.rearrange("p (e m) -> p e m", e=EC),
            in_=wkv_d.rearrange("(e p) m -> p e m", p=128),
        )
        ident = singles.tile([128, 128], bf16)
        nc.sync.dma_start(out=ident, in_=ident_d[:, :])
        wq_sb = singles.tile([128, EC * QD], bf16)  # e-chunk e at cols [e*256,(e+1)*256)
        wo_sb = [singles.tile([128, E], bf16, name=f"wo_sb{kc}")
                 for kc in range(2)]
        # ones on every partition (the broadcast matmul's lhsT sits at
        # partition 64, next to the denominator row)
        onesP = singles.tile([128, 64], bf16)
        nc.vector.memset(onesP, 1.0)

        units = []  # deferred normalize / O-proj units, popped between matmuls
        prev_pair = None

        def emit_x_loads(b_):
            tiles = []
            for e in range(EC):
                xt = xt_pool.tile([128, L], bf16, name=f"xt_{e}", tag="xt")
                nc.sync.dma_start(out=xt,
                                  in_=xT_d[b_, e * 128:(e + 1) * 128, :])
                tiles.append(xt)
            return tiles

        xts_next = {}
        for b in range(B):
            # ---- x load: full batch, 16 e-chunk tiles ----
            xts = xts_next.pop(b) if b in xts_next else emit_x_loads(b)

            # ---- QKV projections (kv first so kdup/v_sb prep overlaps) ----
            qpair = [qt_pool.tile([128, L], bf16, name=f"qpair{p}", tag=f"qpair{p}")
                     for p in range(2)]
            kvT = kv_pool.tile([128, L], bf16, name="kvT")  # K^T rows 0:64, V^T rows 64:128
            kdup = kd_pool.tile([128, L], bf16, name="kdup")  # K^T dup at rows 64:128
            # chunk stride 66 = [V (64) | ones | pad] keeps the DMA- and
            # DVE-written bf16 regions 4-byte-word-disjoint
            VW = D + 2
            v_sb = vsb_pool.tile([128, KT * VW], bf16, name="v_sb")

            def make_qkv_group(m, t, xts_, qpair_, kvT_):
                def emit():
                    pp = ps_sc.tile([128, 1024], f32, name="ps_qkv", tag="sc")
                    for e in range(EC):
                        if m < 2:
                            lhsT = wq_sb[:, e * QD + m * 128:
                                         e * QD + (m + 1) * 128]
                        else:
                            lhsT = wkv_sb[:, e * 2 * D:(e + 1) * 2 * D]
                        for hf in range(2):
                            hs = slice(hf * 64, (hf + 1) * 64)
                            nc.tensor.matmul(
                                pp[:, hf * 512:(hf + 1) * 512],
                                lhsT[hs, :],
                                xts_[e][hs, t * 512:(t + 1) * 512],
                                start=(e == 0), stop=(e == EC - 1),
                            )
                    dst = qpair_[m] if m < 2 else kvT_
                    dsl = dst[:, t * 512:(t + 1) * 512]
                    nc.vector.tensor_copy(dsl, pp[:, 512:1024])
                    nc.vector.tensor_add(dsl, dsl, pp[:, 0:512])
                return emit

            for m in (2, 0, 1):  # kv first, then q head pairs
                for t in range(NT):
                    make_qkv_group(m, t, xts, qpair, kvT)()
                    if units:  # drain previous batch's tail work
                        units.pop(0)()

                if m == 2:
                    # K^T duplicate for the row-tiled scores matmul, and the
                    # ones column of v_sb (denominator trick) — both ahead of
                    # the deferred q/o weight loads in the DMA queue so the
                    # first attention pair isn't blocked
                    nc.sync.dma_start(out=kdup[64:128, :], in_=kvT[0:64, :])
                    ones_bcast = bass.AP(
                        tensor=ones_d[0:1, 0:KT].tensor, offset=0,
                        ap=[[0, 128], [1, KT]])
                    nc.sync.dma_start(
                        out=v_sb.rearrange("p (k c) -> p k c", c=VW)[:, :, D],
                        in_=ones_bcast)
                    if b == 0:
                        nc.sync.dma_start(
                            out=wq_sb.rearrange("p (e m) -> p e m", e=EC),
                            in_=wq_d.rearrange("(e p) m -> p e m", p=128),
                        )
                        for kc in range(2):
                            nc.sync.dma_start(
                                out=wo_sb[kc],
                                in_=wo_d[kc * 128:(kc + 1) * 128, :])
                    # V transpose: (d, tok) -> v_sb (tok, d | ones) blocks.
                    # Done as a normal matmul (V^T_chunk).T @ I_64 so the
                    # PSUM output stays fp32 (bf16 transpose-mode PSUM writes
                    # are broken on TRN2).
                    for kt in range(KT):
                        psv = ps_mm.tile([128, 64], f32, name="ps_vt", tag="mm")
                        nc.tensor.matmul(
                            psv, kvT[64:128, kt * 128:(kt + 1) * 128],
                            ident[64:128, 64:128],
                            start=True, stop=True,
                        )
                        nc.vector.tensor_copy(
                            v_sb[:, kt * VW: kt * VW + D], psv
                        )

            # ---- attention + interleaved output projection ----
            # Normalize + O-proj are deferred into a queue of small units
            # (1-2 PE matmuls each) popped between attn@V steps, so the PE
            # always has ready work while ACT crawls through the exps.
            outT = [ot_pool.tile([128, L], bf16, name=f"outT{p}", tag=f"outT{p}")
                    for p in range(2)]

            def make_normalize(vo, m, odd, qsl, outT_=None):
                outT_ = outT if outT_ is None else outT_
                def emit():
                    # denom row to partition 0 (shifted copies are legal,
                    # the custom approx op only works at base 0)
                    dcp = rec_pool.tile([1, 512], f32, name="dcp", tag="dcp")
                    nc.vector.tensor_copy(dcp, vo[D:D + 1, :])
                    recT = rec_pool.tile([1, 512], f32, name="recT", tag="rec")
                    nc.vector.reciprocal_approx_fast(out=recT, in_=dcp)
                    recB = rec_pool.tile([1, 512], bf16, name="recB",
                                         tag="recB")
                    nc.vector.tensor_copy(recB, recT)
                    bc = ps_mm.tile([64, 512], f32, name="ps_bc", tag="mm")
                    nc.tensor.matmul(
                        bc, onesP[0:1, 0:64], recB, start=True, stop=True,
                    )
                    if not odd:
                        nc.vector.tensor_mul(
                            outT_[m][0:64, qsl], vo[0:D, :], bc
                        )
                    else:
                        ntmp = ntmp_pool.tile([64, 512], bf16, name="ntmp",
                                              tag="ntmp")
                        nc.vector.tensor_mul(ntmp, vo[0:D, :], bc)
                        nc.sync.dma_start(out=outT_[m][64:128, qsl],
                                           in_=ntmp)
                return emit

            def make_oproj_units(b_, qc, outT_=None):
                outT_ = outT if outT_ is None else outT_
                qsl = slice(qc * 512, (qc + 1) * 512)
                sts = {}
                units = []
                for m4 in range(4):
                    for mi in range(4):
                        def u(m4=m4, mi=mi):
                            if mi == 0:
                                sts[m4] = stage_pool.tile(
                                    [128, 2048], bf16, name="st", tag="st")
                            st = sts[m4]
                            mc = m4 * 4 + mi
                            msl = slice(mc * 128, (mc + 1) * 128)
                            ps = ps_mm.tile([128, 512], f32, name="ps_op",
                                            tag="mm")
                            for kc in range(2):
                                nc.tensor.matmul(
                                    ps, wo_sb[kc][:, msl], outT_[kc][:, qsl],
                                    start=(kc == 0), stop=(kc == 1),
                                )
                            nc.vector.tensor_copy(
                                st[:, mi * 512:(mi + 1) * 512], ps)
                            if mi == 3:
                                nc.sync.dma_start(
                                    out=out_d[b_, 512 * m4:512 * (m4 + 1),
                                              qsl].rearrange(
                                        "(c p) q -> p c q", c=4),
                                    in_=st.rearrange("p (c q) -> p c q", c=4),
                                )
                        units.append(u)
                return units

            def finish_pair(pv, vaccs):
                # vo copies free the PSUM banks; normalize + O-proj deferred
                for odd in range(2):
                    vo = vo_pool.tile([128, 512], f32, name=f"vo{odd}",
                                      tag=f"vo{odd}")
                    nc.vector.tensor_copy(vo[0:D + 1, :],
                                          vaccs[odd][0:D + 1, :])
                    units.append(make_normalize(vo, pv["m"], odd, pv["qsl"],
                                                pv["outT"]))
                if pv["m"] == 1:
                    units.extend(make_oproj_units(pv["b"], pv["qc"],
                                                  pv["outT"]))

            for qc in range(NT):
                if qc == 1 and b + 1 < B:
                    # prefetch next batch's x during this batch's attention
                    xts_next[b + 1] = emit_x_loads(b + 1)
                qsl = slice(qc * 512, (qc + 1) * 512)
                for m in range(2):  # head pair (h=2m at part 0:64, h'=2m+1 at 64:128)
                    # scores+exp for THIS pair, attn@V for the PREVIOUS pair,
                    # and one deferred unit share each kt slot so the PE
                    # stream stays in lockstep with ACT's exp cadence
                    pv = prev_pair
                    if pv is not None:
                        vaccs = [ps_va.tile([128, 512], f32,
                                            name=f"ps_vacc{o}", tag="vacc")
                                 for o in range(2)]
                    es_list = []
                    for kt in range(KT):
                        ksl = slice(kt * 128, (kt + 1) * 128)
                        ssc = ps_sc.tile([128, 1024], f32, name="ps_sc", tag="sc")
                        nc.tensor.matmul(
                            ssc[:, 0:512], kvT[0:64, ksl], qpair[m][0:64, qsl],
                            start=True, stop=True,
                        )
                        nc.tensor.matmul(
                            ssc[:, 512:1024], kdup[64:128, ksl],
                            qpair[m][64:128, qsl],
                            start=True, stop=True,
                        )
                        es = es_pool.tile([128, 1024], bf16, name="es", tag="es")
                        nc.scalar.activation(
                            es, ssc, mybir.ActivationFunctionType.Exp, scale=0.125
                        )
                        es_list.append(es)
                        if pv is not None:
                            for odd in range(2):
                                nc.tensor.matmul(
                                    vaccs[odd][0:D + 1, :],
                                    pv["vsb"][:, kt * VW: kt * VW + D + 1],
                                    pv["es"][kt][:, odd * 512:(odd + 1) * 512],
                                    start=(kt == 0), stop=(kt == KT - 1),
                                )
                        if units:
                            units.pop(0)()
                    if pv is not None:
                        finish_pair(pv, vaccs)
                    prev_pair = dict(es=es_list, m=m, qsl=qsl, qc=qc, b=b,
                                     vsb=v_sb, outT=outT)
        # drain the last pair's attn@V and all deferred work
        pv = prev_pair
        vaccs = [ps_va.tile([128, 512], f32, name=f"ps_vacc{o}", tag="vacc")
                 for o in range(2)]
        for kt in range(KT):
            for odd in range(2):
                nc.tensor.matmul(
                    vaccs[odd][0:D + 1, :],
                    pv["vsb"][:, kt * VW: kt * VW + D + 1],
                    pv["es"][kt][:, odd * 512:(odd + 1) * 512],
                    start=(kt == 0), stop=(kt == KT - 1),
                )
            if units:
                units.pop(0)()
        finish_pair(pv, vaccs)
        while units:
            units.pop(0)()
    nc.compile()
    return nc


def _get_nc():
    if "nc" not in _cache:
        _cache["nc"] = _build_nc()
    return _cache["nc"]


def make_in_maps(x, W_Q, W_K, W_V, W_O):
    import ml_dtypes
    bf16 = ml_dtypes.bfloat16

    x = np.asarray(x, np.float32)
    W_Q = np.asarray(W_Q, np.float32)
    W_K = np.asarray(W_K, np.float32)
    W_V = np.asarray(W_V, np.float32)
    W_O = np.asarray(W_O, np.float32)
    xT = np.ascontiguousarray(x.transpose(0, 2, 1)).astype(bf16)
    in_maps = []
    for h in range(N_CORES):
        in_maps.append({
            "xT": xT,
            "wq": np.ascontiguousarray(W_Q[QD * h:QD * (h + 1), :].T).astype(bf16),
            "wkv": np.ascontiguousarray(
                np.concatenate([W_K[D * h:D * (h + 1), :],
                                W_V[D * h:D * (h + 1), :]], axis=0).T).astype(bf16),
            "wo": np.ascontiguousarray(W_O[:, QD * h:QD * (h + 1)].T).astype(bf16),
            "ident": np.eye(128, dtype=np.float32).astype(bf16),
            "ones": np.ones((1, 128), np.float32).astype(bf16),
        })
    return in_maps


def run_spmd(x, W_Q, W_K, W_V, W_O, **spmd_kwargs):
    from concourse.bass_utils import run_bass_kernel_spmd

    nc = _get_nc()
    in_maps = make_in_maps(x, W_Q, W_K, W_V, W_O)
    res = run_bass_kernel_spmd(nc, in_maps, list(range(N_CORES)), **spmd_kwargs)
    total = np.zeros((B, E, L), np.float32)
    for r in res.results:
        total += np.asarray(r["out"]).astype(np.float32)
    out = np.ascontiguousarray(total.transpose(0, 2, 1))
    return out, res


def kernel(x, W_Q, W_K, W_V, W_O):
    out, _ = run_spmd(x, W_Q, W_K, W_V, W_O)
    return out
